# revision 50
# baseline (speedup 1.0000x reference)
"""Causal multi-head attention with RoPE for Trainium2, 8-core SPMD.

Problem: B=2, S=2048, D_MODEL=1024, H=16, HD=64, causal softmax(QK^T/8)V
with interleaved-pair RoPE on q/k, projections Wq/Wk/Wv/Wo.

Sharding (host side): batch x head-group. Core c handles batch b=c//4 and
head group g=c%4 (heads 4g..4g+3, a 256-wide slice of the projection dims).
Each core computes a full [S, D_MODEL] partial of the output (its head
group's contribution through Wo) in bf16; host casts to f32 and sums 4
partials per batch.

Device strategy (all matmuls bf16, fp32 accumulate):
 - PE warm-up: dummy matmuls from t~0 keep the tensor engine's p-state
   ramping through the input-DMA window so real matmuls start at 2.4 GHz
 - input DMAs: wqk/xt chunk pairs for the first s-chunk with wv and the
   packed cos/sin table in the middle, then xt chunk 1; xt chunks 2-3 and
   wo are deferred via tile_wait_until so the qkt4 transposes (which gate
   the score matmuls) reach the shared DMA engines ahead of them
 - cos/sin shipped as one packed [S, 64] table and broadcast on-device
   (stride-0 AP) over the 8 head blocks: 0.25 MB instead of 2 MB
 - host permutes Wq/Wk rows per head to [evens(32) | odds(32)] so RoPE
   reads contiguous blocks; scores are invariant to the permutation
 - Q,K projected in [s, o] layout -> RoPE -> one batched DMA transpose per
   m-tile into qkt4 [128, 4, S]
 - scoresT[k, q] = Kt.T @ Qt per 128-key block, head pairs row-packed on PE
   partitions 0:64/64:128, both heads of a pair in ONE psum tile so each
   score group needs a single Exp on ACT.  Score psum double-buffers over
   TWO [128, 2x512] tiles (2 banks each); projection psum double-buffers
   (ps_qk/ps_v) and V/PV psum has 2 bufs -- 8 banks total.  Causal
   diagonal masked by one [128, 2, 128] broadcast multiply per diag block
   on GPSIMD (SBUF-only; GPSIMD cannot touch PSUM)
 - probs SBUF buffers alternate by chunk parity (pe_a qc0/qc2, pe_b qc1,
   pe3 qc3 carved from the released phase-1 pool), so pv(qc) reads never
   serialize against scores(qc+1) exp writes
 - emission priority: the qkt4 transpose chain (proj QK tiles) is pulled
   maximally forward -- T15 gates the ACT-critical qc3 exp stream, which
   dominates the second half; V tiles, PV blocks and o_proj ride behind
 - PV flipped: out[q, h, hd] with lhsT = probs block [keys, q], rhs =
   [V | 1] [keys, 65]; col 64 accumulates the softmax denominator, so
   normalization is one reciprocal + ONE broadcast tensor-tensor multiply;
   probs-psum copies on DVE
 - tail: the four last PV blocks transpose on the PE (identity matmul into
   a bf16 psum tile) instead of the DMA xbar, halving the chain latency
   into the final o_proj tiles; the last tile's output leaves in three
   pieces on both DMA queues
 - o_proj per q-chunk rotating over the ps/yq banks (freed score banks at
   the tail); evacuations split across DVE/ACT; each output half DMAs out
   as soon as its own evac lands, queues alternating by (m+nb) parity
"""

import numpy as np
import ml_dtypes

B, S, D, H = 2, 2048, 1024, 16
HD = 64
NCORES = 8
HEADS_PER_CORE = 4
GDIM = HEADS_PER_CORE * HD          # 256 projection cols per core
SB = S // 128                        # 16 s-tiles
KD = D // 128                        # 8 k-tiles over d
QCHUNK = 512
NQC = S // QCHUNK                    # 4 q-chunks
GCAP = 512                           # q-cols per score psum group

_BF16 = ml_dtypes.bfloat16
_cache = {}


def _score_layout(qc):
    """Per (qc): block list in emission order, chunked into <=GCAP-col psum
    groups. Returns (groups, base, ncols) where groups is a list of
    [(kb, qoff, n, colbase), ...] and base maps kb -> global pe column."""
    q0 = qc * QCHUNK
    order = list(range(4 * qc)) + [4 * qc, 4 * qc + 1, 4 * qc + 3, 4 * qc + 2]
    base = {}
    blocks = []
    pos = 0
    for kb in order:
        r = max(0, kb - 4 * qc)
        qoff = q0 + r * 128 if kb >= 4 * qc else q0
        n = QCHUNK - r * 128 if kb >= 4 * qc else QCHUNK
        base[kb] = pos
        blocks.append((kb, qoff, n, pos))
        pos += n
    groups, cur, cols = [], [], 0
    for (kb, qoff, n, colbase) in blocks:
        if cols + n > GCAP:
            groups.append(cur)
            cur, cols = [], 0
        cur.append((kb, qoff, n, colbase))
        cols += n
    groups.append(cur)
    return groups, base, pos


def _build(use_rope: bool):
    import concourse.bass as bass
    import concourse.mybir as mybir
    import concourse.tile as tile
    from concourse import bacc
    from contextlib import ExitStack

    F32 = mybir.dt.float32
    BF16 = mybir.dt.bfloat16
    EXP = mybir.ActivationFunctionType.Exp

    nc = bacc.Bacc(None, target_bir_lowering=False)

    xt_d = nc.dram_tensor("xt", [D, S], BF16, kind="ExternalInput")
    wqk_d = nc.dram_tensor("wqk", [D, 2 * GDIM], BF16, kind="ExternalInput")
    wv_d = nc.dram_tensor("wv", [D, GDIM], BF16, kind="ExternalInput")
    wo_d = nc.dram_tensor("wo", [GDIM, D], BF16, kind="ExternalInput")
    cs_d = nc.dram_tensor("cs1", [S, 64], BF16, kind="ExternalInput")
    mask_d = nc.dram_tensor("maskT", [128, 128], BF16, kind="ExternalInput")
    ident_d = nc.dram_tensor("ident", [128, 128], BF16, kind="ExternalInput")
    out_d = nc.dram_tensor("out", [S, D], BF16, kind="ExternalOutput")

    # pe probs buffer column count for the widest chunk (qc=3)
    _, _, NCOLS = _score_layout(NQC - 1)

    xt_dr = xt_d.rearrange("(k p) s -> p k s", p=128)
    wqk_dr = wqk_d.rearrange("(k p) o -> p k o", p=128)
    wv_dr = wv_d.rearrange("(k p) o -> p k o", p=128)
    cs_dr = cs_d.rearrange("(m p) f -> p m f", p=128)

    with tile.TileContext(nc) as tc:
        es = ExitStack()
        big = es.enter_context(tc.tile_pool(name="big", bufs=1))
        work = es.enter_context(tc.tile_pool(name="work", bufs=2))
        scp = es.enter_context(tc.tile_pool(name="sc", bufs=1, space="PSUM"))
        yqp = es.enter_context(tc.tile_pool(name="yq", bufs=2, space="PSUM"))
        pp = es.enter_context(tc.tile_pool(name="pp", bufs=1, space="PSUM"))

        # ---- resident tiles ----
        wo = big.tile([128, 2, D], BF16)
        maskT = big.tile([128, 128], BF16)
        ident = big.tile([128, 128], BF16)
        qkt4 = big.tile([128, 4, S], BF16)
        vsb = big.tile([128, SB, HEADS_PER_CORE * 65], BF16)
        yt2 = big.tile([128, 2, S], BF16)
        # probs buffers for qc0-2 (max 5376 cols per head); qc3 gets its own
        # buffers carved from the released phase-1 pool so exp(qc3) need not
        # wait for PV(qc2) to drain these.  Layout: [128, 2(i), NC] per hp.
        _, _, NC2 = _score_layout(2)
        _, _, NC1 = _score_layout(1)
        pe_a = [big.tile([128, 2, NC2], BF16, tag=f"peA{hp}", name=f"peA{hp}")
                for hp in range(2)]
        pe_b = [big.tile([128, 2, NC1], BF16, tag=f"peB{hp}", name=f"peB{hp}")
                for hp in range(2)]
        wusrc = big.tile([128, 512], BF16)
        # phase-1-only tensors: released after the last projection m-tile
        ph1_ctx = tc.tile_pool(name="ph1", bufs=1)
        ph1 = ph1_ctx.__enter__()
        xt = ph1.tile([128, KD, S], BF16)
        wqk = ph1.tile([128, KD, 2 * GDIM], BF16)
        wv = ph1.tile([128, KD, GDIM], BF16)
        if use_rope:
            cs1 = ph1.tile([128, SB, 64], BF16)

        # ---- PE warm-up: keep the tensor engine busy (and its p-state
        # ramping) while the first input DMAs land. ----
        nc.vector.memset(wusrc[:], 0.0)
        vsb4 = vsb.rearrange("p m (h c) -> p m h c", h=4)
        nc.vector.memset(vsb4[:, :, :, 64:65], 1.0)

        sc_state = {"i": 0}

        def sc_tile(tag=None):
            if tag is None:
                tag = f"sc{sc_state['i'] % 2}"
                sc_state["i"] += 1
            return scp.tile([128, 2 * GCAP], F32, tag=tag, name="sc")

        def warmup(n):
            for _ in range(n):
                wu = sc_tile()
                nc.tensor.matmul(wu[:, 0:512], wusrc[:, 0:128], wusrc[:],
                                 start=True, stop=True)

        warmup(6)

        # ---- input DMAs (SP queue) ----
        for k in range(0, KD, 2):
            nc.sync.dma_start(wqk[:, k:k + 2, :], wqk_dr[:, k:k + 2, :])
            nc.sync.dma_start(xt[:, k:k + 2, 0:QCHUNK], xt_dr[:, k:k + 2, 0:QCHUNK])
        nc.sync.dma_start(wv[:], wv_dr[:])
        if use_rope:
            nc.sync.dma_start(cs1[:], cs_dr[:])
        c1 = slice(QCHUNK, 2 * QCHUNK)
        nc.sync.dma_start(xt[:, :, c1], xt_dr[:, :, c1])
        for c in (2, 3):
            cs = slice(c * QCHUNK, (c + 1) * QCHUNK)
            nc.sync.dma_start(xt[:, :, cs], xt_dr[:, :, cs])
        nc.sync.dma_start(wo[:], wo_d.rearrange("(k p) o -> p k o", p=128))
        nc.scalar.dma_start(maskT[:], mask_d[:])
        nc.scalar.dma_start(ident[:], ident_d[:])

        # ---------- emission helpers ----------
        def proj_qk(m, wu=0):
            """QK projection + rope + transpose for s-tile m."""
            ms = slice(m * 128, (m + 1) * 128)
            ps = pp.tile([128, 2 * GDIM], F32,
                         tag=("ps_qk", "ps_v")[m % 2], name="ps")
            for k in range(KD):
                nc.tensor.matmul(ps[:], xt[:, k, ms], wqk[:, k, :],
                                 start=(k == 0), stop=(k == KD - 1))
                if wu and k % 2 == 1 and k < 2 * wu:
                    warmup(1)
            qkr = work.tile([128, 2 * GDIM], BF16, tag="qkr", name="qkr", bufs=4)
            if use_rope:
                qkf = work.tile([128, 2 * GDIM], BF16, tag="qkf", name="qkf")
                if m < 4:
                    nc.scalar.copy(qkf[:], ps[:])
                else:
                    # split halves across DVE/ACT: halves the psum-bank
                    # hold time and spreads engine load
                    nc.vector.tensor_copy(qkf[:, 0:256], ps[:, 0:256])
                    nc.scalar.copy(qkf[:, 256:512], ps[:, 256:512])
                # head dims are [evens(32) | odds(32)] per 64-block (host
                # permuted): E/O are 8 contiguous 32-col blocks at stride 64
                qv = qkf.rearrange("p (hb eo f) -> p hb eo f", eo=2, f=32)
                ov = qkr.rearrange("p (hb eo f) -> p hb eo f", eo=2, f=32)
                E, O = qv[:, :, 0, :], qv[:, :, 1, :]
                C = cs1[:, m, 0:32].unsqueeze(1).broadcast_to([128, 8, 32])
                Sn = cs1[:, m, 32:64].unsqueeze(1).broadcast_to([128, 8, 32])
                t_c = work.tile([128, 512], BF16, tag="tc", name="tc")
                t_s = work.tile([128, 512], BF16, tag="ts", name="ts")
                tcv = t_c.rearrange("p (hb eo f) -> p hb eo f", eo=2, f=32)
                tsv = t_s.rearrange("p (hb eo f) -> p hb eo f", eo=2, f=32)
                nc.vector.tensor_mul(tcv[:, :, 0, :], E, C)
                nc.vector.tensor_mul(tcv[:, :, 1, :], O, C)
                nc.vector.tensor_mul(tsv[:, :, 0, :], E, Sn)
                nc.vector.tensor_mul(tsv[:, :, 1, :], O, Sn)
                # e' = E*c - O*s ; o' = O*c + E*s
                nc.vector.tensor_sub(ov[:, :, 0, :], tcv[:, :, 0, :], tsv[:, :, 1, :])
                nc.vector.tensor_add(ov[:, :, 1, :], tcv[:, :, 1, :], tsv[:, :, 0, :])
            else:
                nc.vector.tensor_copy(qkr[:], ps[:])
            # one batched transpose: [128 s, 512 o] -> qkt4[:, 0:4, m-block]
            gms = slice(m * 128, (m + 1) * 128)
            nc.sync.dma_start_transpose(qkt4[:, :, gms], qkr[:])

        def proj_v(m):
            """V projection + staging into [V | 1] for s-tile m."""
            ms = slice(m * 128, (m + 1) * 128)
            psv = yqp.tile([128, GDIM], F32, tag="yq", name="psv")
            for k in range(KD):
                nc.tensor.matmul(psv[:], xt[:, k, ms], wv[:, k, :],
                                 start=(k == 0), stop=(k == KD - 1))
            dst = vsb4[:, m, :, 0:64]
            src = psv.rearrange("p (h c) -> p h c", h=4)
            if m < 4:
                nc.scalar.copy(dst, src)
            else:
                nc.vector.tensor_copy(dst, src)

        def score_group(qc, hp, gi, pe_all):
            """Scores + exp + causal mask for group gi of (qc, hp).
            Both heads of the pair go into one psum tile (i0 at cols 0:GCAP,
            i1 at GCAP:2*GCAP) so each group needs a single Exp."""
            groups, base, ncols = _score_layout(qc)
            grp = groups[gi]
            pe2 = pe_all[hp]
            qt = qkt4[:, hp, :]
            kt = qkt4[:, 2 + hp, :]
            gbase = grp[0][3]
            gcols = grp[-1][3] + grp[-1][2] - gbase
            sc = sc_tile()
            for i in range(2):
                rows = slice(i * 64, i * 64 + 64)
                for (kb, qoff, n, colbase) in grp:
                    o = i * GCAP + colbase - gbase
                    nc.tensor.matmul(
                        sc[:, o:o + n],
                        kt[rows, kb * 128:(kb + 1) * 128],
                        qt[rows, qoff:qoff + n],
                        start=True, stop=True)
            scv = sc.rearrange("p (i c) -> p i c", i=2)
            nc.scalar.activation(pe2[:, :, gbase:gbase + gcols],
                                 scv[:, :, 0:gcols], EXP, scale=0.125)
            mb = maskT[:].unsqueeze(1).broadcast_to([128, 2, 128])
            for (kb, qoff, n, colbase) in grp:
                if kb >= 4 * qc:  # diagonal block: causal mask
                    eng = nc.gpsimd
                    eng.tensor_mul(
                        pe2[:, :, colbase:colbase + 128],
                        pe2[:, :, colbase:colbase + 128], mb)

        def pv_block(qc, ql, pe_all, bank="yq", pe_t=False, norm_eng="pool"):
            """Flipped PV for q-block 4*qc+ql: out [128 q, 4 heads, 65], then
            normalize via the accumulated denominator column + transpose.
            pe_t=True transposes on the PE (via identity) instead of the DMA
            xbar -- lower latency for the tail blocks."""
            _, base, _ = _score_layout(qc)
            qb = 4 * qc + ql
            if bank == "yq":
                yq = yqp.tile([128, 4, 65], F32, tag="yq", name="yq")
            else:
                yq = pp.tile([128, 4, 65], F32, tag="ps_qk", name="yq")
            for h in range(4):
                hp, i = divmod(h, 2)
                pe2 = pe_all[hp]
                for kb in range(qb + 1):
                    off = 128 * ql if kb < 4 * qc else 128 * (qb - kb)
                    col = base[kb] + off
                    nc.tensor.matmul(
                        yq[:, h, :], pe2[:, i, col:col + 128],
                        vsb[:, kb, h * 65:(h + 1) * 65],
                        start=(kb == 0), stop=(kb == qb))
            yq_sb = work.tile([128, 4, 65], F32, tag="yqsb", name="yqsb", bufs=3)
            nc.vector.tensor_copy(yq_sb[:], yq[:])
            rc = work.tile([128, 4], F32, tag="rc", name="rc")
            nc.vector.reciprocal(rc[:], yq_sb[:, :, 64])
            y_sb = work.tile([128, 4, 64], BF16, tag="ysb", name="ysb", bufs=3)
            rcb = rc.unsqueeze(2).broadcast_to([128, 4, 64])
            if norm_eng == "pool":
                nc.gpsimd.tensor_mul(y_sb[:], yq_sb[:, :, 0:64], rcb)
            else:
                nc.vector.tensor_mul(y_sb[:], yq_sb[:, :, 0:64], rcb)
            if pe_t:
                ytp = scp.tile([128, 2, 128], BF16,
                               tag="sc1" if ql % 2 else "sc0", name="ytp")
                yv = y_sb.rearrange("p h c -> p (h c)")
                for k in range(2):
                    nc.tensor.transpose(ytp[:, k, :],
                                        yv[:, k * 128:(k + 1) * 128], ident[:])
                nc.scalar.copy(
                    yt2[:, :, qb * 128:(qb + 1) * 128], ytp[:])
            else:
                nc.sync.dma_start_transpose(
                    yt2[:, :, qb * 128:(qb + 1) * 128], y_sb[:])

        def oproj_m(m, tags=("ps_qk", "ps_v"), evac=("dve", "dve")):
            # po rotates over the ps/yq banks (freed score banks at the
            # tail).  Each output half DMAs as soon as its own evacuation
            # lands so no DMA queue head waits on the slower evac engine.
            ms = slice(m * 128, (m + 1) * 128)
            so = work.tile([128, D], BF16, tag="so", name="so", bufs=6)
            for nb in range(2):
                if tags[nb] in ("ps_qk", "ps_v"):
                    po = pp.tile([128, 512], F32, tag=tags[nb], name="po")
                elif tags[nb] == "yq":
                    po = yqp.tile([128, 512], F32, tag="yq", name="po")
                else:
                    po = sc_tile(tag=tags[nb])
                for k in range(2):
                    nc.tensor.matmul(po[:, 0:512], yt2[:, k, ms],
                                     wo[:, k, nb * 512:(nb + 1) * 512],
                                     start=(k == 0), stop=(k == 1))
                dst = so[:, nb * 512:(nb + 1) * 512]
                if evac[nb] == "act":
                    nc.scalar.copy(dst, po[:, 0:512])
                else:
                    nc.vector.tensor_copy(dst, po[:, 0:512])
                eng = nc.scalar if (m + nb) % 2 == 0 else nc.sync
                eng.dma_start(out_d[ms, nb * 512:(nb + 1) * 512], dst)

        # ---------- interleaved emission ----------
        # Score groups alternate with proj/PV/o_proj shadow work; the
        # depth-3 score-psum rotation hides ~1us exp latency per group, so
        # shadows only need to rate-match ACT (~1.1us/group vs 0.43us of
        # PE matmuls per group).  The tile scheduler reorders by readiness
        # within these priorities.
        def interleave(grps, shadows):
            ng_, ns = len(grps), len(shadows)
            si = 0
            for gi, g in enumerate(grps):
                g()
                while si < ns and si + 1 <= (gi + 1) * ns / ng_:
                    shadows[si]()
                    si += 1
            while si < ns:
                shadows[si]()
                si += 1

        def G(qc, hp, gi, pe_all):
            return lambda: score_group(qc, hp, gi, pe_all)

        def PV(qc, ql, pe_all):
            return lambda: pv_block(qc, ql, pe_all)

        def QK(m, wu=0):
            return lambda: proj_qk(m, wu)

        def V(m):
            return lambda: proj_v(m)

        def OP(m, tags=("ps_qk", "ps_v"), evac=("dve", "dve")):
            return lambda: oproj_m(m, tags, evac)

        def ngr(qc):
            return len(_score_layout(qc)[0])

        proj_qk(0, wu=3)
        proj_v(0)
        proj_qk(1, wu=2)
        proj_v(1)
        proj_qk(2, wu=1)
        proj_v(2)
        proj_qk(3, wu=1)
        proj_v(3)
        # probs buffers alternate by chunk parity (pe_a: qc0/qc2, pe_b: qc1,
        # pe3: qc3) so pv(qc) never serializes against scores(qc+1).
        # Shadows for scores(qc) may only contain proj m-tiles whose
        # transposes qc does not read (m >= 4qc+4) and pv blocks of earlier
        # chunks whose probs buffer differs.
        interleave([G(0, 0, i, pe_a) for i in range(ngr(0))] +
                   [G(0, 1, i, pe_a) for i in range(ngr(0))],
                   [QK(4), QK(5), QK(6), QK(7)])
        # qc1 (7 x 2) in pe_b: needs T4-7; V tiles defer behind the
        # T-critical QK chain
        interleave([G(1, 0, i, pe_b) for i in range(ngr(1))] +
                   [G(1, 1, i, pe_b) for i in range(ngr(1))],
                   [QK(8), QK(9), QK(10), QK(11), V(4), V(5), V(6), V(7),
                    V(8), V(9), PV(0, 0, pe_a), PV(0, 1, pe_a)])
        proj_v(10)
        proj_v(11)
        pv_block(0, 2, pe_a)
        pv_block(0, 3, pe_a)
        # qc2 (11 x 2) back in pe_a: needs T8-11 and pv(0) reads done
        qc2g = []
        for i in range(ngr(2)):
            qc2g += [G(2, 0, i, pe_a), G(2, 1, i, pe_a)]
        interleave(qc2g,
                   [QK(12), QK(13), QK(14), QK(15), V(12), V(13),
                    V(14), V(15), PV(1, 0, pe_b), PV(1, 1, pe_b),
                    PV(1, 2, pe_b), PV(1, 3, pe_b)])
        # phase 1 done: free xt/w/cos/sin, carve qc3 probs buffers from the
        # freed region so exp(qc3) is independent of PV(qc2)
        ph1_ctx.__exit__(None, None, None)
        with tc.tile_pool(name="pe3p", bufs=1) as pe3p:
            pe3 = [pe3p.tile([128, 2, NCOLS], BF16, tag=f"pe3{hp}",
                             name=f"pe3{hp}") for hp in range(2)]
            # qc3 (15 x 2) in pe3: needs T12-15; o_proj m needs yt2 block m
            # (pv of that block emitted earlier)
            qc3_groups = []
            for i in range(ngr(3)):
                qc3_groups += [G(3, 0, i, pe3), G(3, 1, i, pe3)]
            interleave(qc3_groups,
                       [PV(2, 0, pe_a), OP(0), OP(1), PV(2, 1, pe_a), OP(2),
                        OP(3), PV(2, 2, pe_a), OP(4), OP(5), OP(6), OP(7),
                        PV(2, 3, pe_a), OP(8), OP(9), OP(10)])
            pv_block(3, 0, pe3, bank="yq", pe_t=True, norm_eng="dve")
            oproj_m(11)
            pv_block(3, 1, pe3, bank="ps", pe_t=True, norm_eng="dve")
            oproj_m(12, tags=("sc0", "yq"), evac=("act", "dve"))
            pv_block(3, 2, pe3, bank="yq", pe_t=True, norm_eng="dve")
            oproj_m(13, tags=("sc1", "ps_v"), evac=("act", "dve"))
            pv_block(3, 3, pe3, bank="ps", pe_t=True, norm_eng="dve")
            oproj_m(14, tags=("sc0", "yq"), evac=("act", "dve"))
            # last tile: nb0 as usual; nb1 quartered across engines/queues
            ms15 = slice(15 * 128, 16 * 128)
            so15 = work.tile([128, D], BF16, tag="so", name="so", bufs=6)
            po = sc_tile(tag="sc1")
            for k in range(2):
                nc.tensor.matmul(po[:, 0:512], yt2[:, k, ms15],
                                 wo[:, k, 0:512], start=(k == 0), stop=(k == 1))
            nc.scalar.copy(so15[:, 0:512], po[:, 0:512])
            nc.sync.dma_start(out_d[ms15, 0:512], so15[:, 0:512])
            po = pp.tile([128, 512], F32, tag="ps_v", name="po")
            for k in range(2):
                nc.tensor.matmul(po[:, 0:512], yt2[:, k, ms15],
                                 wo[:, k, 512:1024], start=(k == 0), stop=(k == 1))
            nc.vector.tensor_copy(so15[:, 512:1024], po[:, 0:512])
            nc.scalar.dma_start(out_d[ms15, 512:768], so15[:, 512:768])
            nc.sync.dma_start(out_d[ms15, 768:1024], so15[:, 768:1024])
        es.close()
    nc.compile()
    return nc


_PERM64 = np.concatenate([np.arange(0, 64, 2), np.arange(1, 64, 2)])


def _prep_core_inputs(x, Wq, Wk, Wv, Wo, cos_g, sin_g, use_rope):
    """Host-side shard + layout prep. Returns list of 8 input dicts."""
    maskT = np.tril(np.ones((128, 128), np.float32)).T.astype(_BF16)
    cs1 = np.concatenate([cos_g, sin_g], axis=1).astype(_BF16)  # [S, 64]
    maps = []
    for c in range(NCORES):
        b, g = divmod(c, HEADS_PER_CORE)
        rows = slice(g * GDIM, (g + 1) * GDIM)
        wq_g = Wq[rows]
        wk_g = Wk[rows]
        if use_rope:
            # per-head row permutation to [evens(32) | odds(32)] so device
            # rope works on contiguous blocks; scores invariant (q,k share it)
            wq_g = wq_g.reshape(HEADS_PER_CORE, HD, D)[:, _PERM64, :].reshape(GDIM, D)
            wk_g = wk_g.reshape(HEADS_PER_CORE, HD, D)[:, _PERM64, :].reshape(GDIM, D)
        wqk = np.concatenate([wq_g, wk_g], axis=0).T  # [D, 512]
        m = {
            "xt": np.ascontiguousarray(x[b].T).astype(_BF16),
            "wqk": np.ascontiguousarray(wqk).astype(_BF16),
            "wv": np.ascontiguousarray(Wv[rows].T).astype(_BF16),
            "wo": np.ascontiguousarray(Wo[:, rows].T).astype(_BF16),
            "maskT": maskT,
            "ident": np.eye(128, dtype=np.float32).astype(_BF16),
        }
        if use_rope:
            m["cs1"] = cs1
        maps.append(m)
    return maps


def kernel(x, token_positions, use_rope, Wq, Wk, Wv, Wo, cos, sin):
    from concourse.bass_utils import run_bass_kernel_spmd

    x = np.asarray(x, np.float32)
    token_positions = np.asarray(token_positions)
    Wq = np.asarray(Wq, np.float32)
    Wk = np.asarray(Wk, np.float32)
    Wv = np.asarray(Wv, np.float32)
    Wo = np.asarray(Wo, np.float32)
    cos = np.asarray(cos, np.float32)
    sin = np.asarray(sin, np.float32)
    rope = bool(int(use_rope))

    cos_g = cos[token_positions]  # [S, 32]
    sin_g = sin[token_positions]

    if rope not in _cache:
        _cache[rope] = _build(rope)
    nc = _cache[rope]

    in_maps = _prep_core_inputs(x, Wq, Wk, Wv, Wo, cos_g, sin_g, rope)
    res = run_bass_kernel_spmd(nc, in_maps, list(range(NCORES)))

    out = np.zeros((B, S, D), np.float32)
    for c in range(NCORES):
        out[c // HEADS_PER_CORE] += res.results[c]["out"].astype(np.float32)
    return out
